# revision 13
# baseline (speedup 1.0000x reference)
"""Bass/Trainium2 kernel for nn_HailNet_42975442763785 (GNN message passing).

Math insight: the COO adjacency built by the model only references node
indices in [0, 4111), so h1 = (A @ xf.T) is zero outside its first 4111
rows and the [48,65536] @ [65536,256] embedding matmul reduces exactly to
[48,4111] @ [4111,256].  A is banded (offsets within [-80, 80]), so
stage A (A.T blocks @ xfT) is a block-tridiagonal matmul over 128-wide
blocks.

v2 strategy (vs the v1 collective kernel): the 8-core AllReduce of the
t2 pre-activations measured ~30us on HW (small-payload collectives have
~15us fixed overhead), dwarfing everything else.  v2 REPLICATES stages
A+B on all 8 cores instead — zero communication — and pays for it with
bf16/fp8 inputs (adjacency values are small integers; only one entry
(17) is not e4m3-exact; host-side study: rel err 3e-5 overall).

GRU restructure: the input projections x_proj for all 12 steps live in
PSUM with the gate biases folded in via tiny K=6/K=2 "bias matmuls"
(lhsT = bias rows, rhs = 0/1 selector), and each step's W_hh @ h
re-accumulates onto the same PSUM region.  The r,z sigmoid then reads
PSUM directly, removing one DVE hop + two bias ops per step from the
serial chain.  Per-step critical path: PE mm -> Act sigmoid -> DVE
(npre, nin) -> Act tanh -> DVE (w, h') -> PE.

All activation column axes are ordered t-major (col = t*B + b) so each
GRU step reads contiguous PSUM columns.
"""

from contextlib import ExitStack

import numpy as np

import concourse.bass as bass
import concourse.tile as tile
from concourse import bacc, mybir
from concourse.bass_utils import run_bass_kernel_spmd

F32 = mybir.dt.float32
BF16 = mybir.dt.bfloat16
AF = mybir.ActivationFunctionType
ALU = mybir.AluOpType

def _view2d(t, n):
    """[P, n] contiguous free-dim view of a tile AP (for tensor_tensor_scan,
    which requires 2-D operands)."""
    from concourse.bass_types import AP
    return AP(t.tensor, t.offset, [list(t.ap[0]), [1, n]])


N_CORES = 8
BLK = 128
NBR = 33                  # replicated I-blocks (33*128 = 4224 >= 4111)
SUP = 4111                # true support of the adjacency
N = 65536
BT, B, T = 48, 4, 12
EMB, HID, G3 = 256, 256, 768
GA = 8                    # stage-A I-blocks per PSUM tile


# ---------------------------------------------------------------- device code

def build_program(repeat: int = 1, loads_in_body: bool = False,
                  at_fp8: bool = False, t_steps: int = T):
    """loads_in_body=True re-DMAs all large inputs every repeat iteration so
    the repeat-slope timing includes the input-streaming cost of a one-shot
    run (this is a memory-regime problem)."""
    nc = bacc.Bacc("TRN2", target_bir_lowering=False, debug=False,
                   num_devices=N_CORES)

    nb = NBR
    A8 = mybir.dt.float8e4 if at_fp8 else BF16
    at_d = nc.dram_tensor("at", [BLK, 3 * nb, BLK], A8, kind="ExternalInput")
    xh_d = nc.dram_tensor("xh", [BLK, nb + 2, BT], BF16, kind="ExternalInput")
    wes_d = nc.dram_tensor("wes", [BLK, nb, EMB], BF16, kind="ExternalInput")
    # wl1 | wih | whh | wf0 packed along the free dim -> one DMA
    WB = EMB + G3 + G3 + 16
    wb_d = nc.dram_tensor("wbig", [BLK, 2, WB], BF16, kind="ExternalInput")
    wf1_d = nc.dram_tensor("wf1t", [16, 16], F32, kind="ExternalInput")
    wf2_d = nc.dram_tensor("wf2t", [16, 1], F32, kind="ExternalInput")
    # bias rows for the K=small bias matmuls + the 0/1 selector
    bias6_d = nc.dram_tensor("bias6", [6, BLK], F32, kind="ExternalInput")
    bhn2_d = nc.dram_tensor("bhn2", [2, BLK], F32, kind="ExternalInput")
    bemb2_d = nc.dram_tensor("bemb2", [2, BLK], F32, kind="ExternalInput")
    bl12_d = nc.dram_tensor("bl12", [2, BLK], F32, kind="ExternalInput")
    sel6_d = nc.dram_tensor("sel6", [6, 6 * BT], F32, kind="ExternalInput")
    h0_d = nc.dram_tensor("h0c", [BLK, 2, B], BF16, kind="ExternalInput")
    bf0_d = nc.dram_tensor("bf0", [16, 1], F32, kind="ExternalInput")
    bf1_d = nc.dram_tensor("bf1", [16, 1], F32, kind="ExternalInput")
    bf2_d = nc.dram_tensor("bf2", [1, 1], F32, kind="ExternalInput")
    out_d = nc.dram_tensor("out", [1, B], F32, kind="ExternalOutput")

    with tile.TileContext(nc) as tc, ExitStack() as ctx:
        const = ctx.enter_context(tc.tile_pool(name="const", bufs=1))
        work = ctx.enter_context(tc.tile_pool(name="work", bufs=2))
        gru = ctx.enter_context(tc.tile_pool(name="gru", bufs=2))
        psA = ctx.enter_context(tc.tile_pool(name="psA", bufs=2, space="PSUM"))
        psM = ctx.enter_context(tc.tile_pool(name="psM", bufs=2, space="PSUM"))
        psX = ctx.enter_context(tc.tile_pool(name="psX", bufs=2, space="PSUM"))
        psH = ctx.enter_context(tc.tile_pool(name="psH", bufs=1, space="PSUM"))

        def emit_loads(pool):
            """DMA the large per-core inputs + weights into SBUF tiles.
            Spread across three HWDGE queues (SP/DVE/Act) and chunk the big
            tensors so stage A/B can start as soon as their slices land."""
            at_sb = pool.tile([BLK, 3 * nb, BLK], A8, tag="at_sb")
            for c0, c1 in ((0, 33), (33, 66), (66, 3 * nb)):
                nc.sync.dma_start(out=at_sb[:, c0:c1, :], in_=at_d[:, c0:c1, :])
            xh_sb = pool.tile([BLK, nb + 2, BT], BF16, tag="xh_sb")
            nc.scalar.dma_start(out=xh_sb[:], in_=xh_d[:])
            wes_sb = pool.tile([BLK, nb, EMB], BF16, tag="wes_sb")
            for c0, c1 in ((0, 17), (17, nb)):
                nc.scalar.dma_start(out=wes_sb[:, c0:c1, :],
                                    in_=wes_d[:, c0:c1, :])
            wb_sb = pool.tile([BLK, 2, WB], BF16, tag="wb_sb")
            nc.gpsimd.dma_start(out=wb_sb[:], in_=wb_d[:])
            wf1_sb = pool.tile([16, 16], F32, tag="wf1_sb")
            nc.gpsimd.dma_start(out=wf1_sb[:], in_=wf1_d[:])
            wf2_sb = pool.tile([16, 1], F32, tag="wf2_sb")
            nc.gpsimd.dma_start(out=wf2_sb[:], in_=wf2_d[:])
            wl1_sb = wb_sb[:, :, 0:EMB]
            wih_sb = wb_sb[:, :, EMB:EMB + G3]
            whh_sb = wb_sb[:, :, EMB + G3:EMB + 2 * G3]
            wf0_sb = wb_sb[:, :, EMB + 2 * G3:WB]
            return at_sb, xh_sb, wes_sb, wl1_sb, wih_sb, whh_sb, \
                wf0_sb, wf1_sb, wf2_sb

        if not loads_in_body:
            (at_sb, xh_sb, wes_sb, wl1_sb, wih_sb, whh_sb,
             wf0_sb, wf1_sb, wf2_sb) = emit_loads(const)
        bias6_sb = const.tile([6, BLK], F32)
        nc.gpsimd.dma_start(out=bias6_sb[:], in_=bias6_d[:])
        bhn2_sb = const.tile([2, BLK], F32)
        nc.gpsimd.dma_start(out=bhn2_sb[:], in_=bhn2_d[:])
        bemb2_sb = const.tile([2, BLK], F32)
        nc.gpsimd.dma_start(out=bemb2_sb[:], in_=bemb2_d[:])
        bl12_sb = const.tile([2, BLK], F32)
        nc.gpsimd.dma_start(out=bl12_sb[:], in_=bl12_d[:])
        sel6_sb = const.tile([6, 6, T, B], F32)
        nc.gpsimd.dma_start(out=sel6_sb[:], in_=sel6_d[:])
        h0_sb = const.tile([BLK, 2, B], BF16)
        nc.gpsimd.dma_start(out=h0_sb[:], in_=h0_d[:])
        bf0_sb = const.tile([16, 1], F32)
        nc.gpsimd.dma_start(out=bf0_sb[:], in_=bf0_d[:])
        bf1_sb = const.tile([16, 1], F32)
        nc.gpsimd.dma_start(out=bf1_sb[:], in_=bf1_d[:])
        bf2_sb = const.tile([1, 1], F32)
        nc.gpsimd.dma_start(out=bf2_sb[:], in_=bf2_d[:])

        # warm the ACT sigmoid/tanh table set while DMAs run
        dummy = const.tile([BLK, 1], F32)
        nc.vector.memset(dummy[:], 0.0)
        dummy2 = const.tile([BLK, 1], F32)
        nc.scalar.activation(dummy2[:], dummy[:], AF.Sigmoid)

        for _ in range(repeat):
            if loads_in_body:
                (at_sb, xh_sb, wes_sb, wl1_sb, wih_sb, whh_sb,
                 wf0_sb, wf1_sb, wf2_sb) = emit_loads(work)

            # ---- stage A: h1T blocks [128, 48] = A.T @ xfT (block tridiag)
            h1_sb = work.tile([BLK, nb, BT], BF16)
            ng = (nb + GA - 1) // GA
            for g in range(ng):
                w = min(GA, nb - g * GA)
                ps = psA.tile([BLK, GA, BT], F32, tag="psA")
                for ii in range(w):
                    i = g * GA + ii
                    for jo in range(3):
                        nc.tensor.matmul(
                            ps[:, ii, :], at_sb[:, 3 * i + jo, :],
                            xh_sb[:, i + jo, :],
                            start=(jo == 0), stop=(jo == 2))
                if g % 2 == 0:
                    nc.vector.tensor_copy(
                        h1_sb[:, g * GA:g * GA + w, :], ps[:, :w, :])
                else:
                    nc.scalar.activation(
                        h1_sb[:, g * GA:g * GA + w, :], ps[:, :w, :],
                        AF.Identity)

            # ---- stage B: t2preT [256, 48] = W_es @ h1 + b_emb
            t2p = psM.tile([BLK, 2, BT], F32, tag="mat")
            nc.tensor.matmul(t2p[:, :, :], bemb2_sb[:],
                             sel6_sb[0:2, 0:2, :, :],
                             start=True, stop=False, skip_group_check=True)
            for e in range(2):
                for i in range(nb):
                    nc.tensor.matmul(
                        t2p[:, e, :], wes_sb[:, i, e * BLK:(e + 1) * BLK],
                        h1_sb[:, i, :], start=False, stop=(i == nb - 1),
                        skip_group_check=True)
            t2_sb = work.tile([BLK, 2, BT], BF16)
            nc.scalar.activation(t2_sb[:], t2p[:], AF.Sigmoid)

            # ---- stage C: t4T = sigmoid(W_l1 @ t2T + b_l1)
            t4p = psM.tile([BLK, 2, BT], F32, tag="mat")
            nc.tensor.matmul(t4p[:, :, :], bl12_sb[:],
                             sel6_sb[0:2, 0:2, :, :],
                             start=True, stop=False, skip_group_check=True)
            for mc in range(2):
                for kc in range(2):
                    nc.tensor.matmul(
                        t4p[:, mc, :], wl1_sb[:, kc, mc * BLK:(mc + 1) * BLK],
                        t2_sb[:, kc, :], start=False, stop=(kc == 1),
                        skip_group_check=True)
            t4_sb = work.tile([BLK, 2, BT], BF16)
            nc.scalar.activation(t4_sb[:], t4p[:], AF.Sigmoid)

            # ---- stage D: x_proj for all steps -> PSUM, biases folded in
            ps_xp = psX.tile([BLK, 6, T, B], F32)
            nc.tensor.matmul(ps_xp[:, :, :, :], bias6_sb[:], sel6_sb[:],
                             start=True, stop=False, skip_group_check=True)
            for c in range(6):
                for kc in range(2):
                    nc.tensor.matmul(
                        ps_xp[:, c, :, :],
                        wih_sb[:, kc, c * BLK:(c + 1) * BLK],
                        t4_sb[:, kc, :], start=False, stop=(kc == 1),
                        skip_group_check=True)
            # xn staged to SBUF once: DVE reads it cheaper there per step
            xn_sb = work.tile([BLK, 2, T, B], F32)
            nc.vector.tensor_copy(xn_sb[:], ps_xp[:, 4:6, :, :])

            # scan operand buffers for the blend h' = (1*(v*nw)) + u:
            # triples (k=0: nw, k=1: *v, k=2: +u) along the free dim, so one
            # tensor_tensor_scan replaces the w=nw*v / h'=w+u pair.
            d0 = work.tile([BLK, 2, B, 3], F32, tag="d0")
            nc.gpsimd.memset(d0[:, :, :, 0:2], 0.0)
            nc.gpsimd.memset(d0[:, :, :, 2], 1.0)
            d1 = work.tile([BLK, 2, B, 3], F32, tag="d1")
            nc.gpsimd.memset(d1[:, :, :, 1], 0.0)

            # ---- GRU over T steps
            h_prev = h0_sb
            for t in range(t_steps):
                # W_hh @ h accumulates onto the x-proj PSUM for r,z ...
                for c in range(4):
                    for kc in range(2):
                        nc.tensor.matmul(
                            ps_xp[:, c, t, :],
                            whh_sb[:, kc, c * BLK:(c + 1) * BLK],
                            h_prev[:, kc, :], start=False, stop=(kc == 1),
                            skip_group_check=True)
                # ... and into a fresh tile (bias pre-folded) for n
                ps_hn = psH.tile([BLK, 2, B], F32, tag="hn")
                nc.tensor.matmul(ps_hn[:, :, :], bhn2_sb[:],
                                 sel6_sb[0:2, 0:2, 0, :],
                                 start=True, stop=False,
                                 skip_group_check=True)
                for cc in range(2):
                    for kc in range(2):
                        nc.tensor.matmul(
                            ps_hn[:, cc, :],
                            whh_sb[:, kc, (4 + cc) * BLK:(5 + cc) * BLK],
                            h_prev[:, kc, :], start=False, stop=(kc == 1),
                            skip_group_check=True)
                rz = gru.tile([BLK, 4, B], F32, tag="rz")
                nc.scalar.activation(rz[:], ps_xp[:, 0:4, t, :], AF.Sigmoid)
                npre = gru.tile([BLK, 2, B], F32, tag="npre")
                nc.vector.tensor_mul(npre[:], ps_hn[:], rz[:, 0:2, :])
                nin = gru.tile([BLK, 2, B], F32, tag="nin")
                nc.vector.tensor_add(nin[:], npre[:], xn_sb[:, :, t, :])
                # tanh lands in the scan's k=0 slot; u = z*h and v = 1-z land
                # in slots k=2 / k=1 from Pool/GpSimd during the tanh
                nc.scalar.activation(d1[:, :, :, 0], nin[:], AF.Tanh)
                nc.gpsimd.tensor_mul(d1[:, :, :, 2], rz[:, 2:4, :], h_prev[:])
                nc.gpsimd.tensor_scalar(d0[:, :, :, 1], rz[:, 2:4, :],
                                        -1.0, 1.0, op0=ALU.mult, op1=ALU.add)
                # one scan computes h' = (1*(v*nw)) + u at k=2 of each triple
                hsc = gru.tile([BLK, 2, B, 3], BF16, tag="h")
                nc.vector.tensor_tensor_scan(
                    _view2d(hsc, 2 * B * 3), _view2d(d0, 2 * B * 3),
                    _view2d(d1, 2 * B * 3), 0.0,
                    op0=ALU.mult, op1=ALU.add)
                h_prev = hsc[:, :, :, 2]

            # ---- tail MLP: [4,256] -> 16 -> 16 -> 1, sigmoid each
            ps_o1 = psH.tile([16, B], F32, tag="mlp")
            for kc in range(2):
                nc.tensor.matmul(ps_o1[:], wf0_sb[:, kc, :], h_prev[:, kc, :],
                                 start=(kc == 0), stop=(kc == 1))
            o1 = work.tile([16, B], F32, tag="o1s")
            nc.scalar.activation(o1[:], ps_o1[:], AF.Sigmoid, bias=bf0_sb[:])
            ps_o2 = psH.tile([16, B], F32, tag="mlp")
            nc.tensor.matmul(ps_o2[:], wf1_sb[:], o1[:], start=True, stop=True)
            o2 = work.tile([16, B], F32, tag="o2s")
            nc.scalar.activation(o2[:], ps_o2[:], AF.Sigmoid, bias=bf1_sb[:])
            ps_o3 = psH.tile([1, B], F32, tag="mlp")
            nc.tensor.matmul(ps_o3[:], wf2_sb[:], o2[:], start=True, stop=True)
            o3 = work.tile([1, B], F32, tag="o3s")
            nc.scalar.activation(o3[:], ps_o3[:], AF.Sigmoid, bias=bf2_sb[:])
            nc.sync.dma_start(out=out_d[:], in_=o3[:])

    nc.compile()
    return nc


# ---------------------------------------------------------------- host side

def prepare_in_maps(x, h0, rows, cols, W_emb, b_emb, W_l1, b_l1,
                    W_ih, W_hh, b_ih, b_hh, W_f0, b_f0, W_f1, b_f1,
                    W_f2, b_f2, at_fp8=False):
    import ml_dtypes
    f32 = np.float32
    bf = ml_dtypes.bfloat16
    a8 = ml_dtypes.float8_e4m3 if at_fp8 else bf
    nb = NBR
    x = np.ascontiguousarray(x, f32)
    assert int(rows.max()) < SUP and int(cols.max()) < SUP

    # dense banded adjacency on its true support (duplicates sum = coalesce)
    A = np.zeros((SUP, SUP), f32)
    np.add.at(A, (np.asarray(rows), np.asarray(cols)), 1.0)

    S_pad = nb * BLK
    ATp = np.zeros((S_pad, S_pad), f32)
    ATp[:SUP, :SUP] = A.T

    # t-major columns: col = t*B + b
    xf = x.reshape(B, T, N).transpose(1, 0, 2).reshape(BT, N)
    XTp = np.zeros(((nb + 2) * BLK, BT), f32)
    XTp[BLK:BLK + SUP] = xf[:, :SUP].T

    WesT = np.zeros((S_pad, EMB), f32)
    WesT[:SUP] = np.asarray(W_emb, f32)[:, :SUP].T

    at = np.zeros((3 * nb, BLK, BLK), f32)
    for i in range(nb):
        for jo in range(3):
            J = i - 1 + jo
            if 0 <= J < nb:
                at[3 * i + jo] = ATp[J * BLK:(J + 1) * BLK,
                                     i * BLK:(i + 1) * BLK]
    xh = np.ascontiguousarray(
        XTp.reshape(nb + 2, BLK, BT).transpose(1, 0, 2))
    wes = np.ascontiguousarray(
        WesT.reshape(nb, BLK, EMB).transpose(1, 0, 2))

    def pm3(w, k):  # [K, M] weight -> partition-major [128, K//128, M]
        return np.ascontiguousarray(
            np.asarray(w, f32).T.reshape(-1, BLK, k).transpose(1, 0, 2))

    bih = np.asarray(b_ih, f32)
    bhh = np.asarray(b_hh, f32)
    # gate biases as K-rows for the bias matmuls: rz rows carry both
    # b_ih+b_hh, n rows carry b_ih only (b_hh_n folded into ps_hn)
    bias6 = np.concatenate([bih[:512] + bhh[:512], bih[512:]]).reshape(6, BLK)
    bhn2 = bhh[512:].reshape(2, BLK)
    sel6 = np.kron(np.eye(6, dtype=f32), np.ones((1, BT), f32))
    h0c = np.ascontiguousarray(
        np.asarray(h0, f32)[0].T.reshape(2, BLK, B).transpose(1, 0, 2))

    wbig = np.concatenate(
        [pm3(W_l1, EMB), pm3(W_ih, G3), pm3(W_hh, G3), pm3(W_f0, 16)],
        axis=2)
    m = dict(
        at=np.ascontiguousarray(at.transpose(1, 0, 2)).astype(a8),
        xh=xh.astype(bf), wes=wes.astype(bf),
        wbig=np.ascontiguousarray(wbig).astype(bf),
        wf1t=np.ascontiguousarray(np.asarray(W_f1, f32).T),
        wf2t=np.ascontiguousarray(np.asarray(W_f2, f32).T),
        bias6=np.ascontiguousarray(bias6),
        bhn2=np.ascontiguousarray(bhn2),
        bemb2=np.asarray(b_emb, f32).reshape(2, BLK),
        bl12=np.asarray(b_l1, f32).reshape(2, BLK),
        sel6=sel6,
        h0c=h0c.astype(bf),
        bf0=np.asarray(b_f0, f32).reshape(16, 1),
        bf1=np.asarray(b_f1, f32).reshape(16, 1),
        bf2=np.asarray(b_f2, f32).reshape(1, 1),
    )
    return [m] * N_CORES


# production configuration for kernel(); test.py reads this too
KERNEL_CONFIG = dict(at_fp8=True)

_CACHE = {}


def kernel(**inputs) -> np.ndarray:
    if "nc" not in _CACHE:
        _CACHE["nc"] = build_program(**KERNEL_CONFIG)
    nc = _CACHE["nc"]
    in_maps = prepare_in_maps(
        **inputs, **{k: v for k, v in KERNEL_CONFIG.items()
                     if k in ("at_fp8",)})
    res = run_bass_kernel_spmd(nc, in_maps, list(range(N_CORES)))
    out = res.results[0]["out"]          # [1, 4]
    return np.ascontiguousarray(out.T.astype(np.float32))  # [4, 1]


if __name__ == "__main__":
    import importlib.util
    spec = importlib.util.spec_from_file_location("reference", "reference.py")
    ref = importlib.util.module_from_spec(spec)
    spec.loader.exec_module(ref)
    inputs = {k: np.asarray(v) for k, v in ref.setup_inputs().items()}
    expected = np.asarray(ref.reference(**inputs))
    got = kernel(**inputs)
    err = np.abs(got - expected).max() / np.abs(expected).max()
    print("expected:", expected.ravel())
    print("got:     ", got.ravel())
    print("Relative error:", err)
